# revision 1
# baseline (speedup 1.0000x reference)
"""Trainium2 Bass kernel for nn_BilinearScorer.

Reference computation (per full input):
    t = text @ W_text.T + b_text            # [B, H]
    v = t @ W_patch                         # [B, PD]
    scores[b, n] = patches[b, n, :] . v[b]  + t[b] . b_patch   # [B, N]

Strategy: data-parallel over batch B across 8 NeuronCores (4 batches/core).
The heavy op (patches . v) is HBM-bandwidth bound (64 MiB patches per core).
Per core:
  - preamble: t^T[h, b] via fused DVE scalar_tensor_tensor rows against
    partition-broadcast text (DMA replication); v rows / bias on the
    TensorEngine (lhsT = t^T column), replicated across partitions with
    ones-vector matmuls (ScalarE copies PSUM->SBUF);
  - main loop: one fused DVE scalar_tensor_tensor per 128-row block
    (patch block * v_bcast, accum_out = dot along the free dim), which
    keeps DVE (~1.3us/block) under the ~358 GB/s DMA pace (~1.46us per
    512 KB block), so the kernel is HBM-DMA-bound. Per-batch bias is
    added once on the [128, 32] score tile before writeback.
Output is written as [BL, 128, 32] (partition-major) and transposed on host.
"""

import os
import sys

import numpy as np

_REPO = "/opt/trn_rl_repo"
if _REPO not in sys.path:
    sys.path.insert(0, _REPO)

B, N, PD, TD, H = 32, 4096, 1024, 768, 512
NCORES = 8
BL = B // NCORES          # batches per core
P = 128                   # partitions
NB = N // P               # 32 n-blocks of 128 rows
JPT = 4                   # n-blocks per DMA tile (2 MiB per DMA)
NT = NB // JPT            # DMA tiles per batch
HC = H // P               # h chunks
TC = TD // P              # text-dim chunks
PATCH_BUFS = 6

_NC_CACHE = {}
LAST_RESULTS = None       # BassKernelResults of the most recent kernel() call


def _build_nc():
    import concourse.bacc as bacc
    import concourse.bass as bass
    import concourse.mybir as mybir
    from concourse.tile import TileContext

    f32 = mybir.dt.float32
    mult = mybir.AluOpType.mult

    nc = bacc.Bacc("TRN2", target_bir_lowering=False, debug=False,
                   num_devices=NCORES)

    patches = nc.dram_tensor("patches", [BL, N, PD], f32, kind="ExternalInput")[:]
    text = nc.dram_tensor("text", [BL, TD], f32, kind="ExternalInput")[:]
    w_patch = nc.dram_tensor("w_patch", [H, PD], f32, kind="ExternalInput")[:]
    b_patch = nc.dram_tensor("b_patch", [H], f32, kind="ExternalInput")[:]
    w_text = nc.dram_tensor("w_text", [H, TD], f32, kind="ExternalInput")[:]
    b_text = nc.dram_tensor("b_text", [H], f32, kind="ExternalInput")[:]
    scores = nc.dram_tensor("scores", [BL, P, NB], f32, kind="ExternalOutput")[:]

    with TileContext(nc) as tc:
        with (
            tc.tile_pool(name="const", bufs=1) as const,
            tc.tile_pool(name="patch", bufs=PATCH_BUFS) as ppool,
            tc.tile_pool(name="psum", bufs=1, space=bass.MemorySpace.PSUM) as psum,
        ):
            # ---- small-tensor loads ----
            wt_sb = []
            for c in range(HC):
                t_ = const.tile([P, TD], f32, name=f"wt{c}")
                nc.sync.dma_start(
                    out=t_[:], in_=w_text.rearrange("(c p) td -> c p td", p=P)[c]
                )
                wt_sb.append(t_)
            bt_sb = const.tile([P, HC], f32, name="bt_sb")
            nc.sync.dma_start(out=bt_sb[:], in_=b_text.rearrange("(c p) -> p c", p=P))
            bp_sb = const.tile([P, HC], f32, name="bp_sb")
            nc.sync.dma_start(out=bp_sb[:], in_=b_patch.rearrange("(c p) -> p c", p=P))
            wp_sb = []
            for c in range(HC):
                t_ = const.tile([P, PD], f32, name=f"wp{c}")
                nc.sync.dma_start(
                    out=t_[:], in_=w_patch.rearrange("(c p) d -> c p d", p=P)[c]
                )
                wp_sb.append(t_)
            # text rows broadcast across partitions (SWDGE replication)
            tx_bc = []
            for b in range(BL):
                t_ = const.tile([P, TD], f32, name=f"txb{b}")
                nc.gpsimd.dma_start(
                    out=t_[:], in_=text[b : b + 1, :].broadcast_to([P, TD])
                )
                tx_bc.append(t_)
            ones128 = const.tile([1, P], f32, name="ones128")
            nc.vector.memset(ones128[:], 1.0)

            # ---- t^T[h, b] = b_text[h] + sum_td W_text[h, td]*text[b, td] ----
            # b-outer so batch 0's t^T column completes after only 4 STTs and
            # the PE v/broadcast chain for batch 0 starts earlier.
            tT_sb = [const.tile([P, BL], f32, name=f"tT{c}") for c in range(HC)]
            prod_t = const.tile([P, TD], f32, name="prod_t")
            for b in range(BL):
                for c in range(HC):
                    nc.vector.scalar_tensor_tensor(
                        out=prod_t[:],
                        in0=wt_sb[c][:, :],
                        scalar=1.0,
                        in1=tx_bc[b][:, :],
                        op0=mult,
                        op1=mult,
                        accum_out=tT_sb[c][:, b : b + 1],
                    )
                    nc.vector.tensor_scalar_add(
                        out=tT_sb[c][:, b : b + 1],
                        in0=tT_sb[c][:, b : b + 1],
                        scalar1=bt_sb[:, c : c + 1],
                    )

            # ---- per-batch v rows + partition broadcast (PE + ACT) ----
            vbc = []
            for b in range(BL):
                v_row = const.tile([1, PD], f32, name=f"v_row{b}", tag="v_row", bufs=2)
                for half in range(PD // 512):
                    v_ps = psum.tile([1, 512], f32, name=f"v_ps{b}_{half}", tag="v_ps")
                    for c in range(HC):
                        nc.tensor.matmul(
                            v_ps[:],
                            lhsT=tT_sb[c][:, b : b + 1],
                            rhs=wp_sb[c][:, half * 512 : (half + 1) * 512],
                            start=(c == 0),
                            stop=(c == HC - 1),
                        )
                    nc.scalar.copy(
                        out=v_row[0:1, half * 512 : (half + 1) * 512], in_=v_ps[:]
                    )
                vb_sb = const.tile([P, PD], f32, name=f"vbc{b}")
                for half in range(PD // 512):
                    vb_ps = psum.tile(
                        [P, 512], f32, name=f"vb_ps{b}_{half}", tag="vb_ps", bufs=2
                    )
                    nc.tensor.matmul(
                        vb_ps[:],
                        lhsT=ones128[:],
                        rhs=v_row[0:1, half * 512 : (half + 1) * 512],
                        start=True,
                        stop=True,
                    )
                    nc.scalar.copy(
                        out=vb_sb[:, half * 512 : (half + 1) * 512], in_=vb_ps[:]
                    )
                vbc.append(vb_sb)

            # ---- per-batch bias rows + broadcast ----
            br_sb = const.tile([1, BL], f32, name="br_sb")
            for b in range(BL):
                br_ps = psum.tile([1, 1], f32, name=f"brp{b}", tag="br_ps")
                for c in range(HC):
                    nc.tensor.matmul(
                        br_ps[:],
                        lhsT=tT_sb[c][:, b : b + 1],
                        rhs=bp_sb[:, c : c + 1],
                        start=(c == 0),
                        stop=(c == HC - 1),
                    )
                nc.scalar.copy(out=br_sb[0:1, b : b + 1], in_=br_ps[:])
            bbc_ps = psum.tile([P, BL], f32, name="bbc_ps", tag="bbc_ps")
            nc.tensor.matmul(
                bbc_ps[:], lhsT=ones128[:], rhs=br_sb[:], start=True, stop=True
            )
            bbc = const.tile([P, BL], f32, name="bbc")
            nc.scalar.copy(out=bbc[:], in_=bbc_ps[:])

            # ---- main loop: fused DVE dot product per 128-row block ----
            # (GpSimd/ACT offload measured WORSE: POOL tensor_tensor costs
            # ~3.3us/block and its shared SBUF port slows DVE STT by ~40%.)
            prod_main = const.tile([P, PD], f32, name="prod_main")
            for b in range(BL):
                sc_sb = const.tile([P, NB], f32, name=f"sc{b}")
                pr = patches[b].rearrange("(t j p) d -> t p j d", p=P, j=JPT)
                for t in range(NT):
                    tile_ = ppool.tile([P, JPT, PD], f32, tag="ptile", name="ptile")
                    dma_eng = nc.sync if (b * NT + t) % 2 == 0 else nc.scalar
                    dma_eng.dma_start(out=tile_[:], in_=pr[t])
                    for j in range(JPT):
                        col = t * JPT + j
                        nc.vector.scalar_tensor_tensor(
                            out=prod_main[:],
                            in0=tile_[:, j, :],
                            scalar=1.0,
                            in1=vbc[b][:, :],
                            op0=mult,
                            op1=mult,
                            accum_out=sc_sb[:, col : col + 1],
                        )
                nc.vector.tensor_scalar_add(
                    out=sc_sb[:, :], in0=sc_sb[:, :], scalar1=bbc[:, b : b + 1]
                )
                nc.sync.dma_start(out=scores[b], in_=sc_sb[:])

    nc.compile()
    return nc


def _get_nc():
    if "nc" not in _NC_CACHE:
        _NC_CACHE["nc"] = _build_nc()
    return _NC_CACHE["nc"]


def _install_profile_shim():
    """Provide antenv.axon_hooks (NTFF profiling over axon) when absent.

    Replicates trn_agent_boot's ctypes hook against libaxon_pjrt.so so
    run_bass_kernel_spmd(trace=True) can capture device profiles."""
    import contextlib
    import ctypes
    import types

    try:
        from antenv.axon_hooks import get_axon_ntff_profile_hook  # noqa: F401
        return
    except ImportError:
        pass

    so_path = "/opt/axon/libaxon_pjrt.so"
    hook = None
    if os.path.exists(so_path):
        lib = ctypes.CDLL(so_path)
        if hasattr(lib, "axon_start_nrt_profile"):
            lib.axon_start_nrt_profile.argtypes = [
                ctypes.POINTER(ctypes.c_int64),
                ctypes.c_size_t,
            ]
            lib.axon_start_nrt_profile.restype = ctypes.c_int64
            lib.axon_stop_nrt_profile.argtypes = [ctypes.c_char_p]
            lib.axon_stop_nrt_profile.restype = ctypes.c_int64

            @contextlib.contextmanager
            def _hook(output_dir, device_ids):
                import jax

                jax.devices()
                if device_ids:
                    ids = (ctypes.c_int64 * len(device_ids))(*device_ids)
                    rc = lib.axon_start_nrt_profile(ids, len(device_ids))
                else:
                    rc = lib.axon_start_nrt_profile(None, 0)
                if rc != 0:
                    raise RuntimeError(f"axon_start_nrt_profile rc={rc}")
                try:
                    yield
                finally:
                    n = lib.axon_stop_nrt_profile(str(output_dir).encode())
                    print(f"ntff profile: {n} file(s) -> {output_dir}",
                          file=sys.stderr)

            hook = _hook

    mod = types.ModuleType("antenv.axon_hooks")
    mod.get_axon_ntff_profile_hook = lambda: hook
    mod.set_axon_ntff_profile_hook = lambda h: None
    sys.modules["antenv.axon_hooks"] = mod


def kernel(**inputs):
    from concourse.bass_utils import run_bass_kernel_spmd

    global LAST_RESULTS

    patches = np.ascontiguousarray(np.asarray(inputs["patches"], dtype=np.float32))
    text = np.ascontiguousarray(np.asarray(inputs["text"], dtype=np.float32))
    w_patch = np.ascontiguousarray(np.asarray(inputs["W_patch"], dtype=np.float32))
    b_patch = np.ascontiguousarray(np.asarray(inputs["b_patch"], dtype=np.float32))
    w_text = np.ascontiguousarray(np.asarray(inputs["W_text"], dtype=np.float32))
    b_text = np.ascontiguousarray(np.asarray(inputs["b_text"], dtype=np.float32))

    nc = _get_nc()
    in_maps = []
    for c in range(NCORES):
        in_maps.append(
            {
                "patches": patches[c * BL : (c + 1) * BL],
                "text": text[c * BL : (c + 1) * BL],
                "w_patch": w_patch,
                "b_patch": b_patch,
                "w_text": w_text,
                "b_text": b_text,
            }
        )

    trace = bool(int(os.environ.get("KERNEL_PROFILE", "0")))
    if trace:
        _install_profile_shim()
        import concourse.bass_utils as _bu

        _bu.upload_artifacts = lambda tmpdir: ""  # no artifact bucket here
    res = run_bass_kernel_spmd(
        nc, in_maps, core_ids=list(range(NCORES)), trace=trace
    )
    LAST_RESULTS = res

    out = np.concatenate(
        [
            np.transpose(res.results[c]["scores"], (0, 2, 1)).reshape(BL, N)
            for c in range(NCORES)
        ],
        axis=0,
    )
    return out



# revision 3
# speedup vs baseline: 1.1271x; 1.1271x over previous
"""Trainium2 Bass kernel for nn_BilinearScorer.

Reference computation (per full input):
    t = text @ W_text.T + b_text            # [B, H]
    v = t @ W_patch                         # [B, PD]
    scores[b, n] = patches[b, n, :] . v[b]  + t[b] . b_patch   # [B, N]

Strategy: data-parallel over batch B across 8 NeuronCores (4 batches/core).
The heavy op (patches . v) is HBM-bandwidth bound (64 MiB patches per core).
Per core:
  - preamble: t^T[h, b] via fused DVE scalar_tensor_tensor rows against
    partition-broadcast text (DMA replication); v rows / bias on the
    TensorEngine (lhsT = t^T column), replicated across partitions with
    ones-vector matmuls (ScalarE copies PSUM->SBUF);
  - main loop: one fused DVE scalar_tensor_tensor per 128-row block
    (patch block * v_bcast, accum_out = dot along the free dim), which
    keeps DVE (~1.3us/block) under the ~358 GB/s DMA pace (~1.46us per
    512 KB block), so the kernel is HBM-DMA-bound. Per-batch bias is
    added once on the [128, 32] score tile before writeback.
Output is written as [BL, 128, 32] (partition-major) and transposed on host.
"""

import os
import sys

import numpy as np

_REPO = "/opt/trn_rl_repo"
if _REPO not in sys.path:
    sys.path.insert(0, _REPO)

B, N, PD, TD, H = 32, 4096, 1024, 768, 512
NCORES = 8
BL = B // NCORES          # batches per core
P = 128                   # partitions
NB = N // P               # 32 n-blocks of 128 rows
JPT = 4                   # n-blocks per DMA tile (2 MiB per DMA)
NT = NB // JPT            # DMA tiles per batch
HC = H // P               # h chunks
TC = TD // P              # text-dim chunks
PATCH_BUFS = 6

_NC_CACHE = {}
LAST_RESULTS = None       # BassKernelResults of the most recent kernel() call


def _build_nc():
    import concourse.bacc as bacc
    import concourse.bass as bass
    import concourse.mybir as mybir
    from concourse.tile import TileContext

    f32 = mybir.dt.float32
    mult = mybir.AluOpType.mult

    nc = bacc.Bacc("TRN2", target_bir_lowering=False, debug=False,
                   num_devices=NCORES)

    patches = nc.dram_tensor("patches", [BL, N, PD], f32, kind="ExternalInput")[:]
    text = nc.dram_tensor("text", [BL, TD], f32, kind="ExternalInput")[:]
    w_patch = nc.dram_tensor("w_patch", [H, PD], f32, kind="ExternalInput")[:]
    b_patch = nc.dram_tensor("b_patch", [H], f32, kind="ExternalInput")[:]
    w_text = nc.dram_tensor("w_text", [H, TD], f32, kind="ExternalInput")[:]
    b_text = nc.dram_tensor("b_text", [H], f32, kind="ExternalInput")[:]
    scores = nc.dram_tensor("scores", [BL, P, NB], f32, kind="ExternalOutput")[:]

    with TileContext(nc) as tc:
        with (
            tc.tile_pool(name="const", bufs=1) as const,
            tc.tile_pool(name="patch", bufs=PATCH_BUFS) as ppool,
            tc.tile_pool(name="psum", bufs=1, space=bass.MemorySpace.PSUM) as psum,
        ):
            # ---- small-tensor loads ----
            wt_sb = []
            for c in range(HC):
                t_ = const.tile([P, TD], f32, name=f"wt{c}")
                nc.sync.dma_start(
                    out=t_[:], in_=w_text.rearrange("(c p) td -> c p td", p=P)[c]
                )
                wt_sb.append(t_)
            bt_sb = const.tile([P, HC], f32, name="bt_sb")
            nc.sync.dma_start(out=bt_sb[:], in_=b_text.rearrange("(c p) -> p c", p=P))
            bp_sb = const.tile([P, HC], f32, name="bp_sb")
            nc.sync.dma_start(out=bp_sb[:], in_=b_patch.rearrange("(c p) -> p c", p=P))
            wp_sb = []
            for c in range(HC):
                t_ = const.tile([P, PD], f32, name=f"wp{c}")
                nc.sync.dma_start(
                    out=t_[:], in_=w_patch.rearrange("(c p) d -> c p d", p=P)[c]
                )
                wp_sb.append(t_)
            # text rows broadcast across partitions (SWDGE replication)
            tx_bc = []
            for b in range(BL):
                t_ = const.tile([P, TD], f32, name=f"txb{b}")
                nc.gpsimd.dma_start(
                    out=t_[:], in_=text[b : b + 1, :].broadcast_to([P, TD])
                )
                tx_bc.append(t_)
            ones128 = const.tile([1, P], f32, name="ones128")
            nc.vector.memset(ones128[:], 1.0)

            # ---- t^T[h, b] = b_text[h] + sum_td W_text[h, td]*text[b, td] ----
            # b-outer so batch 0's t^T column completes after only 4 STTs and
            # the PE v/broadcast chain for batch 0 starts earlier.
            tT_sb = [const.tile([P, BL], f32, name=f"tT{c}") for c in range(HC)]
            prod_t = const.tile([P, TD], f32, name="prod_t")
            for b in range(BL):
                for c in range(HC):
                    nc.vector.scalar_tensor_tensor(
                        out=prod_t[:],
                        in0=wt_sb[c][:, :],
                        scalar=1.0,
                        in1=tx_bc[b][:, :],
                        op0=mult,
                        op1=mult,
                        accum_out=tT_sb[c][:, b : b + 1],
                    )
                    nc.vector.tensor_scalar_add(
                        out=tT_sb[c][:, b : b + 1],
                        in0=tT_sb[c][:, b : b + 1],
                        scalar1=bt_sb[:, c : c + 1],
                    )

            # ---- per-batch v rows + partition broadcast (PE + ACT) ----
            vbc = []
            for b in range(BL):
                v_row = const.tile([1, PD], f32, name=f"v_row{b}", tag="v_row", bufs=2)
                for half in range(PD // 512):
                    v_ps = psum.tile([1, 512], f32, name=f"v_ps{b}_{half}", tag="v_ps")
                    for c in range(HC):
                        nc.tensor.matmul(
                            v_ps[:],
                            lhsT=tT_sb[c][:, b : b + 1],
                            rhs=wp_sb[c][:, half * 512 : (half + 1) * 512],
                            start=(c == 0),
                            stop=(c == HC - 1),
                        )
                    nc.scalar.copy(
                        out=v_row[0:1, half * 512 : (half + 1) * 512], in_=v_ps[:]
                    )
                vb_sb = const.tile([P, PD], f32, name=f"vbc{b}")
                for half in range(PD // 512):
                    vb_ps = psum.tile(
                        [P, 512], f32, name=f"vb_ps{b}_{half}", tag="vb_ps", bufs=2
                    )
                    nc.tensor.matmul(
                        vb_ps[:],
                        lhsT=ones128[:],
                        rhs=v_row[0:1, half * 512 : (half + 1) * 512],
                        start=True,
                        stop=True,
                    )
                    nc.scalar.copy(
                        out=vb_sb[:, half * 512 : (half + 1) * 512], in_=vb_ps[:]
                    )
                vbc.append(vb_sb)

            # ---- per-batch bias rows + broadcast ----
            br_sb = const.tile([1, BL], f32, name="br_sb")
            for b in range(BL):
                br_ps = psum.tile([1, 1], f32, name=f"brp{b}", tag="br_ps")
                for c in range(HC):
                    nc.tensor.matmul(
                        br_ps[:],
                        lhsT=tT_sb[c][:, b : b + 1],
                        rhs=bp_sb[:, c : c + 1],
                        start=(c == 0),
                        stop=(c == HC - 1),
                    )
                nc.scalar.copy(out=br_sb[0:1, b : b + 1], in_=br_ps[:])
            bbc_ps = psum.tile([P, BL], f32, name="bbc_ps", tag="bbc_ps")
            nc.tensor.matmul(
                bbc_ps[:], lhsT=ones128[:], rhs=br_sb[:], start=True, stop=True
            )
            bbc = const.tile([P, BL], f32, name="bbc")
            nc.scalar.copy(out=bbc[:], in_=bbc_ps[:])

            # ---- main loop: fused DVE dot product per 128-row block ----
            # (GpSimd/ACT offload measured WORSE: POOL tensor_tensor costs
            # ~3.3us/block and its shared SBUF port slows DVE STT by ~40%.)
            prod_main = const.tile([P, PD], f32, name="prod_main")
            for b in range(BL):
                sc_sb = const.tile([P, NB], f32, name=f"sc{b}")
                # n = t*512 + p*4 + j: each partition reads one contiguous
                # 16 KiB span per tile (128 descriptors/tile instead of 512)
                pr = patches[b].rearrange("(t p j) d -> t p j d", p=P, j=JPT)
                for t in range(NT):
                    tile_ = ppool.tile([P, JPT, PD], f32, tag="ptile", name="ptile")
                    dma_eng = nc.sync if (b * NT + t) % 2 == 0 else nc.scalar
                    dma_eng.dma_start(out=tile_[:], in_=pr[t])
                    for j in range(JPT):
                        col = t * JPT + j
                        nc.vector.scalar_tensor_tensor(
                            out=prod_main[:],
                            in0=tile_[:, j, :],
                            scalar=1.0,
                            in1=vbc[b][:, :],
                            op0=mult,
                            op1=mult,
                            accum_out=sc_sb[:, col : col + 1],
                        )
                nc.vector.tensor_scalar_add(
                    out=sc_sb[:, :], in0=sc_sb[:, :], scalar1=bbc[:, b : b + 1]
                )
                nc.sync.dma_start(out=scores[b], in_=sc_sb[:])

    nc.compile()
    return nc


def _get_nc():
    if "nc" not in _NC_CACHE:
        _NC_CACHE["nc"] = _build_nc()
    return _NC_CACHE["nc"]


def _install_profile_shim():
    """Provide antenv.axon_hooks (NTFF profiling over axon) when absent.

    Replicates trn_agent_boot's ctypes hook against libaxon_pjrt.so so
    run_bass_kernel_spmd(trace=True) can capture device profiles."""
    import contextlib
    import ctypes
    import types

    try:
        from antenv.axon_hooks import get_axon_ntff_profile_hook  # noqa: F401
        return
    except ImportError:
        pass

    so_path = "/opt/axon/libaxon_pjrt.so"
    hook = None
    if os.path.exists(so_path):
        lib = ctypes.CDLL(so_path)
        if hasattr(lib, "axon_start_nrt_profile"):
            lib.axon_start_nrt_profile.argtypes = [
                ctypes.POINTER(ctypes.c_int64),
                ctypes.c_size_t,
            ]
            lib.axon_start_nrt_profile.restype = ctypes.c_int64
            lib.axon_stop_nrt_profile.argtypes = [ctypes.c_char_p]
            lib.axon_stop_nrt_profile.restype = ctypes.c_int64

            @contextlib.contextmanager
            def _hook(output_dir, device_ids):
                import jax

                jax.devices()
                if device_ids:
                    ids = (ctypes.c_int64 * len(device_ids))(*device_ids)
                    rc = lib.axon_start_nrt_profile(ids, len(device_ids))
                else:
                    rc = lib.axon_start_nrt_profile(None, 0)
                if rc != 0:
                    raise RuntimeError(f"axon_start_nrt_profile rc={rc}")
                try:
                    yield
                finally:
                    n = lib.axon_stop_nrt_profile(str(output_dir).encode())
                    print(f"ntff profile: {n} file(s) -> {output_dir}",
                          file=sys.stderr)

            hook = _hook

    mod = types.ModuleType("antenv.axon_hooks")
    mod.get_axon_ntff_profile_hook = lambda: hook
    mod.set_axon_ntff_profile_hook = lambda h: None
    sys.modules["antenv.axon_hooks"] = mod


def kernel(**inputs):
    from concourse.bass_utils import run_bass_kernel_spmd

    global LAST_RESULTS

    patches = np.ascontiguousarray(np.asarray(inputs["patches"], dtype=np.float32))
    text = np.ascontiguousarray(np.asarray(inputs["text"], dtype=np.float32))
    w_patch = np.ascontiguousarray(np.asarray(inputs["W_patch"], dtype=np.float32))
    b_patch = np.ascontiguousarray(np.asarray(inputs["b_patch"], dtype=np.float32))
    w_text = np.ascontiguousarray(np.asarray(inputs["W_text"], dtype=np.float32))
    b_text = np.ascontiguousarray(np.asarray(inputs["b_text"], dtype=np.float32))

    nc = _get_nc()
    in_maps = []
    for c in range(NCORES):
        in_maps.append(
            {
                "patches": patches[c * BL : (c + 1) * BL],
                "text": text[c * BL : (c + 1) * BL],
                "w_patch": w_patch,
                "b_patch": b_patch,
                "w_text": w_text,
                "b_text": b_text,
            }
        )

    trace = bool(int(os.environ.get("KERNEL_PROFILE", "0")))
    if trace:
        _install_profile_shim()
        import concourse.bass_utils as _bu

        _bu.upload_artifacts = lambda tmpdir: ""  # no artifact bucket here
    res = run_bass_kernel_spmd(
        nc, in_maps, core_ids=list(range(NCORES)), trace=trace
    )
    LAST_RESULTS = res

    # scores[b, p, t*JPT + j] holds n = t*(P*JPT) + p*JPT + j
    out = np.concatenate(
        [
            res.results[c]["scores"]
            .reshape(BL, P, NT, JPT)
            .transpose(0, 2, 1, 3)
            .reshape(BL, N)
            for c in range(NCORES)
        ],
        axis=0,
    )
    return out

